# revision 21
# baseline (speedup 1.0000x reference)
"""Trainium2 Bass kernel for nn_DecoderBlock (dynamic-conv decoder block).

Data-parallel over batch: 16 samples -> 8 cores x 2 samples, PLUS the
kernel-predictor is sharded 8-way over its output columns (kp_sw is 75MB
in f32; replicating it made every core stream the whole thing). Each core
computes the predictor for ALL 16 samples on 1/8 of the output columns,
then one AllToAll redistributes so each core holds the full predictor
output for its own 2 samples.

All weights are cast to bf16 on host (halves DMA); activations are bf16
in SBUF (PSUM accumulation stays fp32). Math per sample
(C=512, G=64, cg=8, H=W=32, S=512, Cout=256):
  dw   = conv3x3(reflect_pad(w), kp_sw) + kp_sb        # sharded across cores
  pw   = pooled @ kp_pw.T + kp_pb                      # sharded across cores
  bias = pooled @ kp_bw.T + kp_bb                      # replicated (tiny)
  xn   = instance_norm(x)
  y    = grouped_dynconv3x3(reflect_pad(xn), dw)       # per-sample weights
  y    = grouped_pointwise(pw, y) + bias
  y    = relu(conv3x3(y, dec_w1) + b1)
  y    = relu(conv3x3(y, dec_w2) + b2)
  out  = nearest_upsample_2x(y)

Grouped convs use block-diagonal [128,128] weight tiles, built on device by
scattering the exchanged predictor output through a DRAM scratch with a
stride-1032 diagonal-embedding view.
"""

import sys

sys.path.insert(0, "/opt/trn_rl_repo")

import numpy as np

import concourse.bacc as bacc
import concourse.bass as bass
import concourse.tile as tile
from concourse import mybir
from concourse.alu_op_type import AluOpType
from concourse.bass_utils import run_bass_kernel_spmd

F32 = mybir.dt.float32
BF16 = mybir.dt.bfloat16
AF = mybir.ActivationFunctionType

NCORES = 8
B = 16           # total batch
BPC = 2          # samples per core
C = 512          # in channels
CO = 256         # out channels
S = 512          # style dim
G = 64           # groups
CG = 8           # channels per group
H = W = 32
HW = H * W
NT = C // 128    # 4 channel tiles
NM2 = CO // 128  # 2 out-channel tiles
EPS = 1e-5
DIAG = CG * 128 + CG          # 1032: row stride of the diagonal-embedding view
SCR = 128 * 129               # 16512 >= 15*1032 + 8*128 + 8 + 1; = dwblk tile + pad
SHARD = C * CG // NCORES      # 512 predictor output columns per core
ROWS = B * 9                  # 144 (sample, kpos) rows of the dw predictor output
CHNK = 9 * BPC + BPC          # 20 rows per all-to-all chunk: 18 dw + 2 pw

_CACHE = {}


def _build():
    nc = bacc.Bacc(None, target_bir_lowering=False, num_devices=NCORES)

    x2 = nc.declare_dram_parameter("x2", [BPC, C, H, W], BF16, isOutput=False)
    wst = nc.declare_dram_parameter("wst", [BPC, S, 3, 3], F32, isOutput=False)
    wfull = nc.declare_dram_parameter("wfull", [B, S, 3, 3], F32, isOutput=False)
    kpswc = nc.declare_dram_parameter("kpswc", [9 * S + 1, SHARD], BF16, isOutput=False)
    kppwc = nc.declare_dram_parameter("kppwc", [S + 1, SHARD], BF16, isOutput=False)
    kpbw = nc.declare_dram_parameter("kpbw", [S + 1, C], BF16, isOutput=False)
    w1t = nc.declare_dram_parameter("w1t", [9, C, C], BF16, isOutput=False)
    w2t = nc.declare_dram_parameter("w2t", [9, C, CO], BF16, isOutput=False)
    b1d = nc.declare_dram_parameter("b1d", [C], F32, isOutput=False)
    b2d = nc.declare_dram_parameter("b2d", [CO], F32, isOutput=False)
    yout = nc.declare_dram_parameter("yout", [BPC, CO, 2 * H, 2 * W], F32, isOutput=True)

    dwscr = nc.dram_tensor("dwscr", [BPC, NT, 9 * SCR], BF16)
    pwscr = nc.dram_tensor("pwscr", [BPC, NT, SCR], BF16)
    ccin = nc.dram_tensor("ccin", [NCORES, CHNK, SHARD], BF16)
    ccout = nc.dram_tensor("ccout", [NCORES, CHNK, SHARD], BF16)

    with tile.TileContext(nc) as tc:
        with (
            tc.tile_pool(name="consts", bufs=1) as consts,
            tc.tile_pool(name="stream", bufs=4) as stream,
            tc.tile_pool(name="streamb", bufs=2) as streamb,
            tc.tile_pool(name="wstream", bufs=3) as wstream,
            tc.tile_pool(name="blk", bufs=2) as blkp,
            tc.tile_pool(name="act", bufs=2) as actp,
            tc.tile_pool(name="pad3", bufs=6) as pad3,
            tc.tile_pool(name="outp", bufs=2) as outp,
            tc.tile_pool(name="psum", bufs=6, space="PSUM") as psum,
            tc.tile_pool(name="psumb", bufs=2, space="PSUM") as psumb,
        ):
            # ---------------- persistent small constants ----------------
            b1sb = consts.tile([128, NT], F32, tag="b1sb")
            nc.sync.dma_start(out=b1sb[:, :], in_=b1d.rearrange("(m c) -> c m", c=128))
            b2sb = consts.tile([128, NM2], F32, tag="b2sb")
            nc.sync.dma_start(out=b2sb[:, :], in_=b2d.rearrange("(m c) -> c m", c=128))

            epsb = consts.tile([128, 1], F32, tag="epsb")
            nc.vector.memset(epsb[:], EPS)
            onesf = consts.tile([1, ROWS], F32, tag="onesf")
            nc.vector.memset(onesf[:], 1.0)
            ones = consts.tile([1, ROWS], BF16, tag="ones")
            nc.vector.tensor_copy(ones[:], onesf[:])
            zpadh = consts.tile([128, 136], BF16, tag="zpadh")
            nc.vector.memset(zpadh[:], 0.0)
            zbig = consts.tile([128, 9 * SCR // 128], BF16, tag="zbig")
            nc.vector.memset(zbig[:], 0.0)

            # ---- x input loads first on the ACT ring (feed instance norm) ----
            xsbs = {}
            for b in range(BPC):
                for t in range(NT):
                    xsb = actp.tile(
                        [128, HW], BF16, tag="xsb", bufs=2 * NT, name=f"xsb{b}{t}"
                    )
                    xsbs[b, t] = xsb
                    nc.scalar.dma_start(
                        out=xsb[:],
                        in_=x2[b, 128 * t : 128 * (t + 1), :, :].rearrange(
                            "c h w -> c (h w)"
                        ),
                    )

            # ------- dwscr/pwscr zero-fill (independent; runs early, ACT ring) -------
            for b in range(BPC):
                for t in range(NT):
                    nc.scalar.dma_start(
                        out=dwscr[b, t].rearrange("(p c) -> p c", p=128),
                        in_=zbig[:, :],
                    )
                    nc.scalar.dma_start(
                        out=pwscr[b, t].rearrange("(p c) -> p c", p=128),
                        in_=zbig[:, :129],
                    )

            # ---------------- full-batch style maps + pooled ----------------
            wsb = consts.tile([128, 4, B, 9], F32, tag="wsb")
            nc.sync.dma_start(
                out=wsb[:, :, :, :],
                in_=wfull.rearrange("b (q p) kh kw -> p q b (kh kw)", q=4),
            )
            pooled = consts.tile([128, 4, B], BF16, tag="pooled")
            pooled_f = consts.tile([128, 4, B], F32, tag="pooledf")
            for q in range(4):
                nc.vector.tensor_reduce(
                    out=pooled_f[:, q, :],
                    in_=wsb[:, q, :, :],
                    axis=mybir.AxisListType.X,
                    op=AluOpType.add,
                )
            nc.scalar.mul(pooled[:, :, :], pooled_f[:, :, :], 1.0 / 9.0)

            # local style pooled (for the replicated bias predictor)
            wsbl = consts.tile([128, 4, BPC, 9], F32, tag="wsbl")
            nc.sync.dma_start(
                out=wsbl[:, :, :, :],
                in_=wst.rearrange("b (q p) kh kw -> p q b (kh kw)", q=4),
            )
            pooledl = consts.tile([128, 4, BPC], BF16, tag="pooledl")
            pooledl_f = consts.tile([128, 4, BPC], F32, tag="pooledlf")
            for q in range(4):
                nc.vector.tensor_reduce(
                    out=pooledl_f[:, q, :],
                    in_=wsbl[:, q, :, :],
                    axis=mybir.AxisListType.X,
                    op=AluOpType.add,
                )
            nc.scalar.mul(pooledl[:, :, :], pooledl_f[:, :, :], 1.0 / 9.0)

            # ---------------- reflect-pad all 16 style maps ----------------
            wp = consts.tile([128, 4, B, 5, 5], BF16, tag="wp")
            for q in range(4):
                w3 = wsb[:, q, :, :].rearrange("p b (kh kw) -> p b kh kw", kh=3)
                nc.vector.tensor_copy(wp[:, q, :, 1:4, 1:4], w3)
                nc.vector.tensor_copy(wp[:, q, :, 1:4, 0:1], w3[:, :, :, 1:2])
                nc.vector.tensor_copy(wp[:, q, :, 1:4, 4:5], w3[:, :, :, 1:2])
                nc.vector.tensor_copy(wp[:, q, :, 0, :], wp[:, q, :, 2, :])
                nc.vector.tensor_copy(wp[:, q, :, 4, :], wp[:, q, :, 2, :])

            # im2col of padded style maps: [128, 9pos, 4q, 144] lhsT chunks
            xw = consts.tile([128, 9, 4, ROWS], BF16, tag="xw")
            for di in range(3):
                for dj in range(3):
                    for q in range(4):
                        nc.vector.tensor_copy(
                            xw[:, di * 3 + dj, q, :].rearrange(
                                "p (b i j) -> p b i j", b=B, i=3
                            ),
                            wp[:, q, :, di : di + 3, dj : dj + 3],
                        )

            # ---------------- predictor: pw (grouped 1x1), sharded ----------------
            ps2 = psumb.tile([B, SHARD], F32, tag="mmb")
            for q in range(4):
                kt = stream.tile([128, SHARD], BF16, tag="kst")
                nc.sync.dma_start(out=kt[:], in_=kppwc[128 * q : 128 * (q + 1), :])
                nc.tensor.matmul(
                    ps2[:], pooled[:, q, :], kt[:], start=(q == 0), stop=False
                )
            rb2 = streamb.tile([1, SHARD], BF16, tag="ksb")
            nc.sync.dma_start(out=rb2[:], in_=kppwc[S : S + 1, :])
            nc.tensor.matmul(ps2[:], ones[:1, :B], rb2[:], start=False, stop=True)
            pwall = consts.tile([B, SHARD], BF16, tag="pwall")
            nc.scalar.mul(pwall[:], ps2[:], 1.0)
            for d in range(NCORES):
                nc.sync.dma_start(
                    out=ccin[d, 9 * BPC : CHNK, :], in_=pwall[BPC * d : BPC * (d + 1), :]
                )

            # ---------------- predictor: dw = kp_sw * wpad (+sb), sharded ----------------
            ps0 = psum.tile([128, SHARD], F32, tag="mm", name="dwps0")
            ps1 = psumb.tile([ROWS - 128, SHARD], F32, tag="mmb")
            first = True
            for pos in range(9):
                for q in range(4):
                    krow = pos * S + 128 * q
                    rt = stream.tile([128, SHARD], BF16, tag="kst")
                    nc.sync.dma_start(out=rt[:], in_=kpswc[krow : krow + 128, :])
                    nc.tensor.matmul(
                        ps0[:], xw[:, pos, q, 0:128], rt[:], start=first, stop=False
                    )
                    nc.tensor.matmul(
                        ps1[:], xw[:, pos, q, 128:ROWS], rt[:], start=first, stop=False
                    )
                    first = False
            rb = streamb.tile([1, SHARD], BF16, tag="ksb")
            nc.sync.dma_start(out=rb[:], in_=kpswc[9 * S : 9 * S + 1, :])
            nc.tensor.matmul(ps0[:], ones[:1, 0:128], rb[:], start=False, stop=True)
            nc.tensor.matmul(ps1[:], ones[:1, 128:ROWS], rb[:], start=False, stop=True)
            dwall0 = consts.tile([128, SHARD], BF16, tag="dwall0")
            nc.scalar.mul(dwall0[:], ps0[:], 1.0)
            dwall1 = consts.tile([ROWS - 128, SHARD], BF16, tag="dwall1")
            nc.scalar.mul(dwall1[:], ps1[:], 1.0)
            # rows (b_global, pos) b-major -> chunk d rows 18d..18d+18
            for d in range(7):
                nc.sync.dma_start(
                    out=ccin[d, 0 : 9 * BPC, :], in_=dwall0[18 * d : 18 * (d + 1), :]
                )
            nc.sync.dma_start(out=ccin[7, 0:2, :], in_=dwall0[126:128, :])
            nc.sync.dma_start(out=ccin[7, 2 : 9 * BPC, :], in_=dwall1[:, :])

            # ---------------- all-to-all: redistribute predictor output ----------------
            nc.gpsimd.collective_compute(
                "AllToAll",
                AluOpType.bypass,
                replica_groups=[list(range(NCORES))],
                ins=[ccin[:, :, :]],
                outs=[ccout[:, :, :]],
            )

            # ------- instance norm: stats for all (b, t), then applies -------
            xps = []
            mvs, rstds = {}, {}
            for b in range(BPC):
                xps.append(pad3.tile([128, NT, 34, 34], BF16, tag="padbuf", name=f"xp{b}"))
            for b in range(BPC):
                for t in range(NT):
                    xsb = xsbs[b, t]
                    st = actp.tile([128, 2, 6], F32, tag="bnst", bufs=2 * NT, name=f"st{b}{t}")
                    xsb2 = xsb[:].rearrange("p (s f) -> p s f", f=512)
                    for sg in range(2):
                        nc.vector.bn_stats(out=st[:, sg, :], in_=xsb2[:, sg, :])
                    mv = actp.tile([128, 2], F32, tag="bnmv", bufs=2 * NT, name=f"mv{b}{t}")
                    mvs[b, t] = mv
                    nc.vector.bn_aggr(out=mv[:], in_=st[:])
                    rstd = actp.tile([128, 1], F32, tag="rstd", bufs=2 * NT, name=f"rstd{b}{t}")
                    rstds[b, t] = rstd
                    nc.scalar.activation(
                        out=rstd[:], in_=mv[:, 1:2], func=AF.Sqrt, bias=epsb[:], scale=1.0
                    )
            for b in range(BPC):
                for t in range(NT):
                    nc.vector.reciprocal(out=rstds[b, t][:], in_=rstds[b, t][:])
            for b in range(BPC):
                xp = xps[b]
                for t in range(NT):
                    nc.vector.tensor_scalar(
                        out=xp[:, t, 1:33, 1:33],
                        in0=xsbs[b, t][:].rearrange("p (h w) -> p h w", h=H),
                        scalar1=mvs[b, t][:, 0:1],
                        scalar2=rstds[b, t][:],
                        op0=AluOpType.subtract,
                        op1=AluOpType.mult,
                    )
                nc.vector.tensor_copy(xp[:, :, 1:33, 0:1], xp[:, :, 1:33, 2:3])
                nc.vector.tensor_copy(xp[:, :, 1:33, 33:34], xp[:, :, 1:33, 31:32])
                nc.vector.tensor_copy(xp[:, :, 0, :], xp[:, :, 2, :])
                nc.vector.tensor_copy(xp[:, :, 33, :], xp[:, :, 31, :])

            # ---------------- predictor: bias (replicated, local samples) ----------------
            biasc = consts.tile([128, NT, BPC], F32, tag="biasc")
            for m in range(NT):
                psb = psumb.tile([128, BPC], F32, tag="mmb")
                rt = stream.tile([128, 4, 128], BF16, tag="kbw")
                nc.sync.dma_start(
                    out=rt[:],
                    in_=kpbw[0:S, 128 * m : 128 * (m + 1)].rearrange(
                        "(q p) j -> p q j", q=4
                    ),
                )
                for q in range(4):
                    nc.tensor.matmul(
                        psb[:], rt[:, q, :], pooledl[:, q, :], start=(q == 0), stop=False
                    )
                rbb = streamb.tile([1, 128], BF16, tag="kbb")
                nc.sync.dma_start(out=rbb[:], in_=kpbw[S : S + 1, 128 * m : 128 * (m + 1)])
                nc.tensor.matmul(psb[:], rbb[:], ones[:1, :BPC], start=False, stop=True)
                nc.scalar.mul(biasc[:, m, :], psb[:], 1.0)

            # ---------------- phase A: adaconv ----------------
            yp1s = []
            for b in range(BPC):
                yp1 = pad3.tile([128, NT, 34, 34], BF16, tag="padbuf", name=f"yp1{b}")
                yp1s.append(yp1)
                nc.vector.tensor_copy(
                    yp1[:, :, 0, :], zpadh[:, :136].rearrange("p (a c) -> p a c", a=4)
                )
                nc.vector.tensor_copy(
                    yp1[:, :, 33, :], zpadh[:, :136].rearrange("p (a c) -> p a c", a=4)
                )
                nc.vector.tensor_copy(
                    yp1[:, :, 1:33, 0:1],
                    zpadh[:, :128].rearrange("p (a b c) -> p a b c", a=4, b=32),
                )
                nc.vector.tensor_copy(
                    yp1[:, :, 1:33, 33:34],
                    zpadh[:, :128].rearrange("p (a b c) -> p a b c", a=4, b=32),
                )
            for b in range(BPC):
                xp = xps[b]
                yp1 = yp1s[b]
                for t in range(NT):
                    # scatter exchanged dw/pw into block-diagonal DRAM scratch.
                    # global predictor col o = t*1024 + g*64 + i*8 + co lives on
                    # source core src = 2t + g//8 at col (g%8)*64 + i*8 + co.
                    for pos in range(9):
                        nc.scalar.dma_start(
                            out=dwscr[b, t]
                            .rearrange("(pos g r) -> pos g r", pos=9, g=16)[
                                pos, :, :1024
                            ]
                            .rearrange("g (i c) -> g i c", i=CG)[:, :, :CG],
                            in_=ccout[2 * t : 2 * t + 2, 9 * b + pos, :].rearrange(
                                "s (gl i co) -> s gl i co", gl=8, i=CG
                            ),
                        )
                    nc.scalar.dma_start(
                        out=pwscr[b, t]
                        .rearrange("(g r) -> g r", g=16)[:, :1024]
                        .rearrange("g (i c) -> g i c", i=CG)[:, :, :CG],
                        in_=ccout[2 * t : 2 * t + 2, 9 * BPC + b, :].rearrange(
                            "s (gl i co) -> s gl i co", gl=8, i=CG
                        ),
                    )
                    dwb = blkp.tile([128, 9, 128], BF16, tag="dwb")
                    nc.scalar.dma_start(
                        out=dwb[:, :, :],
                        in_=dwscr[b, t].rearrange("(pos p c) -> p pos c", pos=9, p=129)[
                            :128, :, :128
                        ],
                    )
                    pwb = blkp.tile([128, 128], BF16, tag="pwb")
                    nc.scalar.dma_start(
                        out=pwb[:],
                        in_=pwscr[b, t, : 128 * 128].rearrange("(p c) -> p c", p=128),
                    )
                    ysb = actp.tile([128, HW], BF16, tag="ysb")
                    for hh in range(2):
                        ps = psum.tile([128, 512], F32, tag="mm")
                        for kdi in range(3):
                            for kdj in range(3):
                                pos = kdi * 3 + kdj
                                nc.tensor.matmul(
                                    ps[:],
                                    dwb[:, pos, :],
                                    xp[:, t, kdi + 16 * hh : kdi + 16 * hh + 16, kdj : kdj + 32],
                                    start=(pos == 0),
                                    stop=(pos == 8),
                                )
                        nc.vector.tensor_copy(ysb[:, 512 * hh : 512 * (hh + 1)], ps[:])
                    for hh in range(2):
                        ps2b = psum.tile([128, 512], F32, tag="mm")
                        nc.tensor.matmul(
                            ps2b[:],
                            pwb[:],
                            ysb[:, 512 * hh : 512 * (hh + 1)],
                            start=True,
                            stop=True,
                        )
                        nc.scalar.activation(
                            out=yp1[:, t, 1 + 16 * hh : 17 + 16 * hh, 1:33],
                            in_=ps2b[:].rearrange("p (h w) -> p h w", h=16),
                            func=AF.Identity,
                            bias=biasc[:, t, b : b + 1],
                            scale=1.0,
                        )

            # ---------------- phase B: conv1 (512 -> 512) + relu ----------------
            yp2s = []
            for b in range(BPC):
                yp2 = pad3.tile([128, NT, 34, 34], BF16, tag="padbuf", name=f"yp2{b}")
                yp2s.append(yp2)
                nc.vector.tensor_copy(
                    yp2[:, :, 0, :], zpadh[:, :136].rearrange("p (a c) -> p a c", a=4)
                )
                nc.vector.tensor_copy(
                    yp2[:, :, 33, :], zpadh[:, :136].rearrange("p (a c) -> p a c", a=4)
                )
                nc.vector.tensor_copy(
                    yp2[:, :, 1:33, 0:1],
                    zpadh[:, :128].rearrange("p (a b c) -> p a b c", a=4, b=32),
                )
                nc.vector.tensor_copy(
                    yp2[:, :, 1:33, 33:34],
                    zpadh[:, :128].rearrange("p (a b c) -> p a b c", a=4, b=32),
                )
            for m in range(NT):
                pss = [psum.tile([128, 512], F32, tag="mm", name=f"pss{i}") for i in range(2 * BPC)]
                for k in range(NT):
                    w1k = wstream.tile([128, 9, 128], BF16, tag="ws")
                    nc.sync.dma_start(
                        out=w1k[:, :, :],
                        in_=w1t[:, 128 * k : 128 * (k + 1), 128 * m : 128 * (m + 1)].rearrange(
                            "pos p j -> p pos j"
                        ),
                    )
                    for b in range(BPC):
                        for hh in range(2):
                            ps = pss[2 * b + hh]
                            for kdi in range(3):
                                for kdj in range(3):
                                    pos = kdi * 3 + kdj
                                    nc.tensor.matmul(
                                        ps[:],
                                        w1k[:, pos, :],
                                        yp1s[b][:, k, kdi + 16 * hh : kdi + 16 * hh + 16, kdj : kdj + 32],
                                        start=(k == 0 and pos == 0),
                                        stop=(k == NT - 1 and pos == 8),
                                    )
                for b in range(BPC):
                    for hh in range(2):
                        nc.scalar.activation(
                            out=yp2s[b][:, m, 1 + 16 * hh : 17 + 16 * hh, 1:33],
                            in_=pss[2 * b + hh][:].rearrange("p (h w) -> p h w", h=16),
                            func=AF.Relu,
                            bias=b1sb[:, m : m + 1],
                            scale=1.0,
                        )

            # ------- phase C: conv2 (512 -> 256) + relu + 2x upsample -------
            for m2 in range(NM2):
                pss = [psum.tile([128, 512], F32, tag="mm", name=f"pss{i}") for i in range(2 * BPC)]
                for k in range(NT):
                    w2k = wstream.tile([128, 9, 128], BF16, tag="ws")
                    nc.sync.dma_start(
                        out=w2k[:, :, :],
                        in_=w2t[:, 128 * k : 128 * (k + 1), 128 * m2 : 128 * (m2 + 1)].rearrange(
                            "pos p j -> p pos j"
                        ),
                    )
                    for b in range(BPC):
                        for hh in range(2):
                            ps = pss[2 * b + hh]
                            for kdi in range(3):
                                for kdj in range(3):
                                    pos = kdi * 3 + kdj
                                    nc.tensor.matmul(
                                        ps[:],
                                        w2k[:, pos, :],
                                        yp2s[b][:, k, kdi + 16 * hh : kdi + 16 * hh + 16, kdj : kdj + 32],
                                        start=(k == 0 and pos == 0),
                                        stop=(k == NT - 1 and pos == 8),
                                    )
                for b in range(BPC):
                    for hh in range(2):
                        ps = pss[2 * b + hh]
                        ous = outp.tile([128, 16, 64], F32, tag="ous")
                        for a in range(2):
                            nc.scalar.activation(
                                out=ous.rearrange("p h (w two) -> p h w two", two=2)[
                                    :, :, :, a
                                ],
                                in_=ps[:].rearrange("p (h w) -> p h w", h=16),
                                func=AF.Relu,
                                bias=b2sb[:, m2 : m2 + 1],
                                scale=1.0,
                            )
                        for a2 in range(2):
                            nc.sync.dma_start(
                                out=yout[b, 128 * m2 : 128 * (m2 + 1), :, :].rearrange(
                                    "c (h two) w -> c h two w", two=2
                                )[:, 16 * hh : 16 * (hh + 1), a2, :],
                                in_=ous[:],
                            )

    nc.compile()
    return nc


def _repack(inputs):
    kp_sw = np.ascontiguousarray(inputs["kp_sw"], dtype=np.float32)
    kp_sb = np.ascontiguousarray(inputs["kp_sb"], dtype=np.float32)
    kp_pw = np.ascontiguousarray(inputs["kp_pw"], dtype=np.float32)
    kp_pb = np.ascontiguousarray(inputs["kp_pb"], dtype=np.float32)
    kp_bw = np.ascontiguousarray(inputs["kp_bw"], dtype=np.float32)
    kp_bb = np.ascontiguousarray(inputs["kp_bb"], dtype=np.float32)
    dec_w1 = np.ascontiguousarray(inputs["dec_w1"], dtype=np.float32)
    dec_b1 = np.ascontiguousarray(inputs["dec_b1"], dtype=np.float32)
    dec_w2 = np.ascontiguousarray(inputs["dec_w2"], dtype=np.float32)
    dec_b2 = np.ascontiguousarray(inputs["dec_b2"], dtype=np.float32)

    import ml_dtypes

    bf16 = ml_dtypes.bfloat16

    # column permutation p' = (t, g, i, co) -> o = (t*128 + g*8 + co)*8 + i
    O = np.arange(C * CG).reshape(NT, 16, CG, CG)  # indexed (t, g, co, i), o-major
    P = O.transpose(0, 1, 3, 2).reshape(-1)        # (t, g, i, co)

    kpsw = np.empty((9 * S + 1, C * CG), dtype=np.float32)
    kpsw[: 9 * S] = (
        kp_sw[P].reshape(C * CG, S, 3, 3).transpose(2, 3, 1, 0).reshape(9 * S, C * CG)
    )  # rows in k-order (di, dj, s)
    kpsw[9 * S] = kp_sb[P]

    # pw column order: o_pw = (g_glob, co, i) flat -> p' = (t, g, i, co)
    O2 = np.arange(C * CG).reshape(NT, 16, CG, CG)  # (t, g, co, i), opw-major
    P2 = O2.transpose(0, 1, 3, 2).reshape(-1)
    kppw = np.empty((S + 1, C * CG), dtype=np.float32)
    kppw[:S] = kp_pw[P2].T
    kppw[S] = kp_pb[P2]

    kpbw = np.empty((S + 1, C), dtype=np.float32)
    kpbw[:S] = kp_bw.T
    kpbw[S] = kp_bb

    w1t = np.ascontiguousarray(dec_w1.transpose(2, 3, 1, 0).reshape(9, C, C))
    w2t = np.ascontiguousarray(dec_w2.transpose(2, 3, 1, 0).reshape(9, C, CO))

    shared = {
        "kpbw": kpbw.astype(bf16),
        "w1t": w1t.astype(bf16),
        "w2t": w2t.astype(bf16),
        "b1d": dec_b1,
        "b2d": dec_b2,
    }
    kpsw_h = kpsw.astype(bf16)
    kppw_h = kppw.astype(bf16)
    percore = []
    for c in range(NCORES):
        sl = slice(SHARD * c, SHARD * (c + 1))
        percore.append(
            {
                "kpswc": np.ascontiguousarray(kpsw_h[:, sl]),
                "kppwc": np.ascontiguousarray(kppw_h[:, sl]),
            }
        )
    return shared, percore


def _make_in_maps(x, w, repacked):
    shared, percore = repacked
    in_maps = []
    for c in range(NCORES):
        sl = slice(BPC * c, BPC * (c + 1))
        in_maps.append(
            {"x2": x[sl], "wst": w[sl], "wfull": w, **shared, **percore[c]}
        )
    return in_maps


def kernel(**inputs):
    if "nc" not in _CACHE:
        _CACHE["nc"] = _build()
    nc = _CACHE["nc"]

    repacked = _repack(inputs)
    x = np.ascontiguousarray(inputs["x"], dtype=np.float32)
    w = np.ascontiguousarray(inputs["w"], dtype=np.float32)

    in_maps = _make_in_maps(x, w, repacked)
    res = run_bass_kernel_spmd(nc, in_maps, list(range(NCORES))).results
    return np.concatenate([r["yout"] for r in res], axis=0)


# revision 31
# speedup vs baseline: 1.0125x; 1.0125x over previous
"""Trainium2 Bass kernel for nn_DecoderBlock (dynamic-conv decoder block).

Data-parallel over batch: 16 samples -> 8 cores x 2 samples, PLUS the
kernel-predictor is sharded 8-way over its output columns (kp_sw is 75MB
in f32; replicating it made every core stream the whole thing). Each core
computes the predictor for ALL 16 samples on 1/8 of the output columns,
then one AllToAll redistributes so each core holds the full predictor
output for its own 2 samples.

All weights are cast to bf16 on host (halves DMA); activations are bf16
in SBUF (PSUM accumulation stays fp32). Math per sample
(C=512, G=64, cg=8, H=W=32, S=512, Cout=256):
  dw   = conv3x3(reflect_pad(w), kp_sw) + kp_sb        # sharded across cores
  pw   = pooled @ kp_pw.T + kp_pb                      # sharded across cores
  bias = pooled @ kp_bw.T + kp_bb                      # replicated (tiny)
  xn   = instance_norm(x)
  y    = grouped_dynconv3x3(reflect_pad(xn), dw)       # per-sample weights
  y    = grouped_pointwise(pw, y) + bias
  y    = relu(conv3x3(y, dec_w1) + b1)
  y    = relu(conv3x3(y, dec_w2) + b2)
  out  = nearest_upsample_2x(y)

Grouped convs use block-diagonal [128,128] weight tiles, built on device by
scattering the exchanged predictor output through a DRAM scratch with a
stride-1032 diagonal-embedding view.
"""

import sys

sys.path.insert(0, "/opt/trn_rl_repo")

import numpy as np

import concourse.bacc as bacc
import concourse.bass as bass
import concourse.tile as tile
from concourse import mybir
from concourse.alu_op_type import AluOpType
from concourse.bass_utils import run_bass_kernel_spmd

F32 = mybir.dt.float32
BF16 = mybir.dt.bfloat16
AF = mybir.ActivationFunctionType

NCORES = 8
B = 16           # total batch
BPC = 2          # samples per core
C = 512          # in channels
CO = 256         # out channels
S = 512          # style dim
G = 64           # groups
CG = 8           # channels per group
H = W = 32
HW = H * W
NT = C // 128    # 4 channel tiles
NM2 = CO // 128  # 2 out-channel tiles
EPS = 1e-5
DIAG = CG * 128 + CG          # 1032: row stride of the diagonal-embedding view
SCR = 128 * 129               # 16512 >= 15*1032 + 8*128 + 8 + 1; = dwblk tile + pad
SHARD = C * CG // NCORES      # 512 predictor output columns per core
ROWS = B * 9                  # 144 (sample, kpos) rows of the dw predictor output
CHNK = 9 * BPC + BPC          # 20 rows per all-to-all chunk: 18 dw + 2 pw

_CACHE = {}


def _build():
    nc = bacc.Bacc(None, target_bir_lowering=False, num_devices=NCORES)

    x2 = nc.declare_dram_parameter("x2", [BPC, C, H, W], BF16, isOutput=False)
    wst = nc.declare_dram_parameter("wst", [BPC, S, 3, 3], F32, isOutput=False)
    wfull = nc.declare_dram_parameter("wfull", [B, S, 3, 3], F32, isOutput=False)
    kpswc = nc.declare_dram_parameter("kpswc", [9 * S + 1, SHARD], BF16, isOutput=False)
    kppwc = nc.declare_dram_parameter("kppwc", [S + 1, SHARD], BF16, isOutput=False)
    kpbw = nc.declare_dram_parameter("kpbw", [S + 1, C], BF16, isOutput=False)
    w1t = nc.declare_dram_parameter("w1t", [9, C, C], BF16, isOutput=False)
    w2t = nc.declare_dram_parameter("w2t", [9, C, CO], BF16, isOutput=False)
    b1d = nc.declare_dram_parameter("b1d", [C], F32, isOutput=False)
    b2d = nc.declare_dram_parameter("b2d", [CO], F32, isOutput=False)
    yout = nc.declare_dram_parameter("yout", [BPC, CO, 2 * H, 2 * W], F32, isOutput=True)

    dwscr = nc.dram_tensor("dwscr", [BPC, NT, 9 * SCR], BF16)
    pwscr = nc.dram_tensor("pwscr", [BPC, NT, SCR], BF16)
    ccin = nc.dram_tensor("ccin", [NCORES, CHNK, SHARD], BF16)
    ccout = nc.dram_tensor("ccout", [NCORES, CHNK, SHARD], BF16)

    with tile.TileContext(nc) as tc:
        with (
            tc.tile_pool(name="consts", bufs=1) as consts,
            tc.tile_pool(name="stream", bufs=4) as stream,
            tc.tile_pool(name="streamb", bufs=2) as streamb,
            tc.tile_pool(name="wstream", bufs=3) as wstream,
            tc.tile_pool(name="blk", bufs=2) as blkp,
            tc.tile_pool(name="act", bufs=2) as actp,
            tc.tile_pool(name="pad3", bufs=6) as pad3,
            tc.tile_pool(name="outp", bufs=2) as outp,
            tc.tile_pool(name="psum", bufs=6, space="PSUM") as psum,
            tc.tile_pool(name="psumb", bufs=2, space="PSUM") as psumb,
        ):
            # ---------------- persistent small constants ----------------
            b1sb = consts.tile([128, NT], F32, tag="b1sb")
            nc.sync.dma_start(out=b1sb[:, :], in_=b1d.rearrange("(m c) -> c m", c=128))
            b2sb = consts.tile([128, NM2], F32, tag="b2sb")
            nc.sync.dma_start(out=b2sb[:, :], in_=b2d.rearrange("(m c) -> c m", c=128))

            epsb = consts.tile([128, 1], F32, tag="epsb")
            nc.vector.memset(epsb[:], EPS)
            onesf = consts.tile([1, ROWS], F32, tag="onesf")
            nc.vector.memset(onesf[:], 1.0)
            ones = consts.tile([1, ROWS], BF16, tag="ones")
            nc.vector.tensor_copy(ones[:], onesf[:])
            zpadh = consts.tile([128, 136], BF16, tag="zpadh")
            nc.vector.memset(zpadh[:], 0.0)
            zbig = consts.tile([128, 9 * SCR // 128], BF16, tag="zbig")
            nc.vector.memset(zbig[:], 0.0)

            # ---- x input loads first on the ACT ring (feed instance norm) ----
            xsbs = {}
            for b in range(BPC):
                for t in range(NT):
                    xsb = actp.tile(
                        [128, HW], BF16, tag="xsb", bufs=2 * NT, name=f"xsb{b}{t}"
                    )
                    xsbs[b, t] = xsb
                    nc.scalar.dma_start(
                        out=xsb[:],
                        in_=x2[b, 128 * t : 128 * (t + 1), :, :].rearrange(
                            "c h w -> c (h w)"
                        ),
                    )

            # ------- dwscr/pwscr zero-fill (independent; runs early, ACT ring) -------
            for b in range(BPC):
                for t in range(NT):
                    nc.scalar.dma_start(
                        out=dwscr[b, t].rearrange("(p c) -> p c", p=128),
                        in_=zbig[:, :],
                    )
                    nc.scalar.dma_start(
                        out=pwscr[b, t].rearrange("(p c) -> p c", p=128),
                        in_=zbig[:, :129],
                    )

            # ---------------- full-batch style maps + pooled ----------------
            wsb = consts.tile([128, B, 4, 9], F32, tag="wsb")
            nc.sync.dma_start(
                out=wsb[:, :, :, :],
                in_=wfull.rearrange("b (q p) kh kw -> p b q (kh kw)", q=4),
            )
            pooled = consts.tile([128, 4, B], BF16, tag="pooled")
            pooled_f = consts.tile([128, 4, B], F32, tag="pooledf")
            for q in range(4):
                nc.vector.tensor_reduce(
                    out=pooled_f[:, q, :],
                    in_=wsb[:, :, q, :],
                    axis=mybir.AxisListType.X,
                    op=AluOpType.add,
                )
            nc.scalar.mul(pooled[:, :, :], pooled_f[:, :, :], 1.0 / 9.0)

            # local style pooled (for the replicated bias predictor)
            wsbl = consts.tile([128, BPC, 4, 9], F32, tag="wsbl")
            nc.sync.dma_start(
                out=wsbl[:, :, :, :],
                in_=wst.rearrange("b (q p) kh kw -> p b q (kh kw)", q=4),
            )
            pooledl = consts.tile([128, 4, BPC], BF16, tag="pooledl")
            pooledl_f = consts.tile([128, 4, BPC], F32, tag="pooledlf")
            for q in range(4):
                nc.vector.tensor_reduce(
                    out=pooledl_f[:, q, :],
                    in_=wsbl[:, :, q, :],
                    axis=mybir.AxisListType.X,
                    op=AluOpType.add,
                )
            nc.scalar.mul(pooledl[:, :, :], pooledl_f[:, :, :], 1.0 / 9.0)

            # ---------------- reflect-pad all 16 style maps ----------------
            wp = consts.tile([128, 4, B, 5, 5], BF16, tag="wp")
            for q in range(4):
                w3 = wsb[:, :, q, :].rearrange("p b (kh kw) -> p b kh kw", kh=3)
                nc.vector.tensor_copy(wp[:, q, :, 1:4, 1:4], w3)
                nc.vector.tensor_copy(wp[:, q, :, 1:4, 0:1], w3[:, :, :, 1:2])
                nc.vector.tensor_copy(wp[:, q, :, 1:4, 4:5], w3[:, :, :, 1:2])
                nc.vector.tensor_copy(wp[:, q, :, 0, :], wp[:, q, :, 2, :])
                nc.vector.tensor_copy(wp[:, q, :, 4, :], wp[:, q, :, 2, :])

            # im2col of padded style maps: [128, 9pos, 4q, 144] lhsT chunks
            xw = consts.tile([128, 9, 4, ROWS], BF16, tag="xw")
            for di in range(3):
                for dj in range(3):
                    for q in range(4):
                        nc.vector.tensor_copy(
                            xw[:, di * 3 + dj, q, :].rearrange(
                                "p (b i j) -> p b i j", b=B, i=3
                            ),
                            wp[:, q, :, di : di + 3, dj : dj + 3],
                        )

            # ---------------- predictor: pw (grouped 1x1), sharded ----------------
            ps2 = psumb.tile([B, SHARD], F32, tag="mmb")
            kt = stream.tile([128, 4, SHARD], BF16, tag="kst")
            nc.sync.dma_start(
                out=kt[:, :, :],
                in_=kppwc[0:S, :].rearrange("(q p) j -> p q j", q=4),
            )
            for q in range(4):
                nc.tensor.matmul(
                    ps2[:], pooled[:, q, :], kt[:, q, :], start=(q == 0), stop=False
                )
            rb2 = streamb.tile([1, SHARD], BF16, tag="ksb")
            nc.sync.dma_start(out=rb2[:], in_=kppwc[S : S + 1, :])
            nc.tensor.matmul(ps2[:], ones[:1, :B], rb2[:], start=False, stop=True)
            pwall = consts.tile([B, SHARD], BF16, tag="pwall")
            nc.scalar.mul(pwall[:], ps2[:], 1.0)
            nc.sync.dma_start(out=ccin[:, 9 * BPC : CHNK, :], in_=pwall[:, :])

            # ---------------- predictor: dw = kp_sw * wpad (+sb), sharded ----------------
            ps0 = psum.tile([128, SHARD], F32, tag="mm", name="dwps0")
            ps1 = psumb.tile([ROWS - 128, SHARD], F32, tag="mmb")
            first = True
            for pos in range(9):
                rt = stream.tile([128, 4, SHARD], BF16, tag="kst")
                nc.sync.dma_start(
                    out=rt[:, :, :],
                    in_=kpswc[pos * S : (pos + 1) * S, :].rearrange(
                        "(q p) j -> p q j", q=4
                    ),
                )
                for q in range(4):
                    nc.tensor.matmul(
                        ps0[:], xw[:, pos, q, 0:128], rt[:, q, :], start=first, stop=False
                    )
                    nc.tensor.matmul(
                        ps1[:], xw[:, pos, q, 128:ROWS], rt[:, q, :], start=first, stop=False
                    )
                    first = False
            rb = streamb.tile([1, SHARD], BF16, tag="ksb")
            nc.sync.dma_start(out=rb[:], in_=kpswc[9 * S : 9 * S + 1, :])
            nc.tensor.matmul(ps0[:], ones[:1, 0:128], rb[:], start=False, stop=True)
            nc.tensor.matmul(ps1[:], ones[:1, 128:ROWS], rb[:], start=False, stop=True)
            dwall0 = consts.tile([128, SHARD], BF16, tag="dwall0")
            nc.scalar.mul(dwall0[:], ps0[:], 1.0)
            dwall1 = consts.tile([ROWS - 128, SHARD], BF16, tag="dwall1")
            nc.scalar.mul(dwall1[:], ps1[:], 1.0)
            # rows (b_global, pos) b-major -> chunk d rows 18d..18d+18
            nc.sync.dma_start(out=ccin[0:7, 0 : 9 * BPC, :], in_=dwall0[0:126, :])
            nc.sync.dma_start(out=ccin[7, 0:2, :], in_=dwall0[126:128, :])
            nc.sync.dma_start(out=ccin[7, 2 : 9 * BPC, :], in_=dwall1[:, :])

            # ---------------- all-to-all: redistribute predictor output ----------------
            nc.gpsimd.collective_compute(
                "AllToAll",
                AluOpType.bypass,
                replica_groups=[list(range(NCORES))],
                ins=[ccin[:, :, :]],
                outs=[ccout[:, :, :]],
            )

            # ------- instance norm: stats for all (b, t), then applies -------
            xps = []
            mvs, rstds = {}, {}
            for b in range(BPC):
                xps.append(pad3.tile([128, NT, 34, 34], BF16, tag="padbuf", name=f"xp{b}"))
            for b in range(BPC):
                for t in range(NT):
                    xsb = xsbs[b, t]
                    st = actp.tile([128, 2, 6], F32, tag="bnst", bufs=2 * NT, name=f"st{b}{t}")
                    xsb2 = xsb[:].rearrange("p (s f) -> p s f", f=512)
                    for sg in range(2):
                        nc.vector.bn_stats(out=st[:, sg, :], in_=xsb2[:, sg, :])
                    mv = actp.tile([128, 2], F32, tag="bnmv", bufs=2 * NT, name=f"mv{b}{t}")
                    mvs[b, t] = mv
                    nc.vector.bn_aggr(out=mv[:], in_=st[:])
                    rstd = actp.tile([128, 1], F32, tag="rstd", bufs=2 * NT, name=f"rstd{b}{t}")
                    rstds[b, t] = rstd
                    nc.scalar.activation(
                        out=rstd[:], in_=mv[:, 1:2], func=AF.Sqrt, bias=epsb[:], scale=1.0
                    )
            for b in range(BPC):
                for t in range(NT):
                    nc.vector.reciprocal(out=rstds[b, t][:], in_=rstds[b, t][:])
            for b in range(BPC):
                xp = xps[b]
                for t in range(NT):
                    nc.vector.tensor_scalar(
                        out=xp[:, t, 1:33, 1:33],
                        in0=xsbs[b, t][:].rearrange("p (h w) -> p h w", h=H),
                        scalar1=mvs[b, t][:, 0:1],
                        scalar2=rstds[b, t][:],
                        op0=AluOpType.subtract,
                        op1=AluOpType.mult,
                    )
                nc.vector.tensor_copy(xp[:, :, 1:33, 0:1], xp[:, :, 1:33, 2:3])
                nc.vector.tensor_copy(xp[:, :, 1:33, 33:34], xp[:, :, 1:33, 31:32])
                nc.vector.tensor_copy(xp[:, :, 0, :], xp[:, :, 2, :])
                nc.vector.tensor_copy(xp[:, :, 33, :], xp[:, :, 31, :])

            # ---------------- predictor: bias (replicated, local samples) ----------------
            biasc = consts.tile([128, NT, BPC], F32, tag="biasc")
            for m in range(NT):
                psb = psumb.tile([128, BPC], F32, tag="mmb")
                rt = stream.tile([128, 4, 128], BF16, tag="kbw")
                nc.sync.dma_start(
                    out=rt[:],
                    in_=kpbw[0:S, 128 * m : 128 * (m + 1)].rearrange(
                        "(q p) j -> p q j", q=4
                    ),
                )
                for q in range(4):
                    nc.tensor.matmul(
                        psb[:], rt[:, q, :], pooledl[:, q, :], start=(q == 0), stop=False
                    )
                rbb = streamb.tile([1, 128], BF16, tag="kbb")
                nc.sync.dma_start(out=rbb[:], in_=kpbw[S : S + 1, 128 * m : 128 * (m + 1)])
                nc.tensor.matmul(psb[:], rbb[:], ones[:1, :BPC], start=False, stop=True)
                nc.scalar.mul(biasc[:, m, :], psb[:], 1.0)

            # ---------------- phase A: adaconv ----------------
            yp1s = []
            for b in range(BPC):
                yp1 = pad3.tile([128, NT, 34, 34], BF16, tag="padbuf", name=f"yp1{b}")
                yp1s.append(yp1)
                nc.vector.tensor_copy(
                    yp1[:, :, 0, :], zpadh[:, :136].rearrange("p (a c) -> p a c", a=4)
                )
                nc.vector.tensor_copy(
                    yp1[:, :, 33, :], zpadh[:, :136].rearrange("p (a c) -> p a c", a=4)
                )
                nc.vector.tensor_copy(
                    yp1[:, :, 1:33, 0:1],
                    zpadh[:, :128].rearrange("p (a b c) -> p a b c", a=4, b=32),
                )
                nc.vector.tensor_copy(
                    yp1[:, :, 1:33, 33:34],
                    zpadh[:, :128].rearrange("p (a b c) -> p a b c", a=4, b=32),
                )
            for b in range(BPC):
                xp = xps[b]
                yp1 = yp1s[b]
                for t in range(NT):
                    # scatter exchanged dw/pw into block-diagonal DRAM scratch.
                    # global predictor col o = t*1024 + g*64 + i*8 + co lives on
                    # source core src = 2t + g//8 at col (g%8)*64 + i*8 + co.
                    for pos in range(9):
                        eng = nc.scalar if pos % 2 == 0 else nc.sync
                        eng.dma_start(
                            out=dwscr[b, t]
                            .rearrange("(pos g r) -> pos g r", pos=9, g=16)[
                                pos, :, :1024
                            ]
                            .rearrange("g (i c) -> g i c", i=CG)[:, :, :CG],
                            in_=ccout[2 * t : 2 * t + 2, 9 * b + pos, :].rearrange(
                                "s (gl i co) -> s gl i co", gl=8, i=CG
                            ),
                        )
                    nc.scalar.dma_start(
                        out=pwscr[b, t]
                        .rearrange("(g r) -> g r", g=16)[:, :1024]
                        .rearrange("g (i c) -> g i c", i=CG)[:, :, :CG],
                        in_=ccout[2 * t : 2 * t + 2, 9 * BPC + b, :].rearrange(
                            "s (gl i co) -> s gl i co", gl=8, i=CG
                        ),
                    )
                    dwb = blkp.tile([128, 9, 128], BF16, tag="dwb")
                    nc.sync.dma_start(
                        out=dwb[:, :, :],
                        in_=dwscr[b, t].rearrange("(pos p c) -> p pos c", pos=9, p=129)[
                            :128, :, :128
                        ],
                    )
                    pwb = blkp.tile([128, 128], BF16, tag="pwb")
                    nc.scalar.dma_start(
                        out=pwb[:],
                        in_=pwscr[b, t, : 128 * 128].rearrange("(p c) -> p c", p=128),
                    )
                    ysb = actp.tile([128, HW], BF16, tag="ysb")
                    for hh in range(2):
                        ps = psum.tile([128, 512], F32, tag="mm")
                        for kdi in range(3):
                            for kdj in range(3):
                                pos = kdi * 3 + kdj
                                nc.tensor.matmul(
                                    ps[:],
                                    dwb[:, pos, :],
                                    xp[:, t, kdi + 16 * hh : kdi + 16 * hh + 16, kdj : kdj + 32],
                                    start=(pos == 0),
                                    stop=(pos == 8),
                                )
                        nc.vector.tensor_copy(ysb[:, 512 * hh : 512 * (hh + 1)], ps[:])
                    for hh in range(2):
                        ps2b = psum.tile([128, 512], F32, tag="mm")
                        nc.tensor.matmul(
                            ps2b[:],
                            pwb[:],
                            ysb[:, 512 * hh : 512 * (hh + 1)],
                            start=True,
                            stop=True,
                        )
                        nc.scalar.activation(
                            out=yp1[:, t, 1 + 16 * hh : 17 + 16 * hh, 1:33],
                            in_=ps2b[:].rearrange("p (h w) -> p h w", h=16),
                            func=AF.Identity,
                            bias=biasc[:, t, b : b + 1],
                            scale=1.0,
                        )

            # ---------------- phase B: conv1 (512 -> 512) + relu ----------------
            yp2s = []
            for b in range(BPC):
                yp2 = pad3.tile([128, NT, 34, 34], BF16, tag="padbuf", name=f"yp2{b}")
                yp2s.append(yp2)
                nc.vector.tensor_copy(
                    yp2[:, :, 0, :], zpadh[:, :136].rearrange("p (a c) -> p a c", a=4)
                )
                nc.vector.tensor_copy(
                    yp2[:, :, 33, :], zpadh[:, :136].rearrange("p (a c) -> p a c", a=4)
                )
                nc.vector.tensor_copy(
                    yp2[:, :, 1:33, 0:1],
                    zpadh[:, :128].rearrange("p (a b c) -> p a b c", a=4, b=32),
                )
                nc.vector.tensor_copy(
                    yp2[:, :, 1:33, 33:34],
                    zpadh[:, :128].rearrange("p (a b c) -> p a b c", a=4, b=32),
                )
            for m in range(NT):
                pss = [psum.tile([128, 512], F32, tag="mm", name=f"pss{i}") for i in range(2 * BPC)]
                for k in range(NT):
                    w1k = wstream.tile([128, 9, 128], BF16, tag="ws")
                    nc.sync.dma_start(
                        out=w1k[:, :, :],
                        in_=w1t[:, 128 * k : 128 * (k + 1), 128 * m : 128 * (m + 1)].rearrange(
                            "pos p j -> p pos j"
                        ),
                    )
                    for b in range(BPC):
                        for hh in range(2):
                            ps = pss[2 * b + hh]
                            for kdi in range(3):
                                for kdj in range(3):
                                    pos = kdi * 3 + kdj
                                    nc.tensor.matmul(
                                        ps[:],
                                        w1k[:, pos, :],
                                        yp1s[b][:, k, kdi + 16 * hh : kdi + 16 * hh + 16, kdj : kdj + 32],
                                        start=(k == 0 and pos == 0),
                                        stop=(k == NT - 1 and pos == 8),
                                    )
                for b in range(BPC):
                    for hh in range(2):
                        nc.scalar.activation(
                            out=yp2s[b][:, m, 1 + 16 * hh : 17 + 16 * hh, 1:33],
                            in_=pss[2 * b + hh][:].rearrange("p (h w) -> p h w", h=16),
                            func=AF.Relu,
                            bias=b1sb[:, m : m + 1],
                            scale=1.0,
                        )

            # ------- phase C: conv2 (512 -> 256) + relu + 2x upsample -------
            for m2 in range(NM2):
                pss = [psum.tile([128, 512], F32, tag="mm", name=f"pss{i}") for i in range(2 * BPC)]
                for k in range(NT):
                    w2k = wstream.tile([128, 9, 128], BF16, tag="ws")
                    nc.sync.dma_start(
                        out=w2k[:, :, :],
                        in_=w2t[:, 128 * k : 128 * (k + 1), 128 * m2 : 128 * (m2 + 1)].rearrange(
                            "pos p j -> p pos j"
                        ),
                    )
                    for b in range(BPC):
                        for hh in range(2):
                            ps = pss[2 * b + hh]
                            for kdi in range(3):
                                for kdj in range(3):
                                    pos = kdi * 3 + kdj
                                    nc.tensor.matmul(
                                        ps[:],
                                        w2k[:, pos, :],
                                        yp2s[b][:, k, kdi + 16 * hh : kdi + 16 * hh + 16, kdj : kdj + 32],
                                        start=(k == 0 and pos == 0),
                                        stop=(k == NT - 1 and pos == 8),
                                    )
                for b in range(BPC):
                    for hh in range(2):
                        ps = pss[2 * b + hh]
                        ous = outp.tile([128, 16, 2, 64], F32, tag="ous")
                        for a in range(2):
                            nc.scalar.activation(
                                out=ous[:, :, 0, :].rearrange(
                                    "p h (w two) -> p h w two", two=2
                                )[:, :, :, a],
                                in_=ps[:].rearrange("p (h w) -> p h w", h=16),
                                func=AF.Relu,
                                bias=b2sb[:, m2 : m2 + 1],
                                scale=1.0,
                            )
                        nc.vector.tensor_copy(ous[:, :, 1, :], ous[:, :, 0, :])
                        nc.sync.dma_start(
                            out=yout[b, 128 * m2 : 128 * (m2 + 1), :, :].rearrange(
                                "c (h two) w -> c h (two w)", two=2
                            )[:, 16 * hh : 16 * (hh + 1), :],
                            in_=ous[:],
                        )

    nc.compile()
    return nc


def _repack(inputs):
    kp_sw = np.ascontiguousarray(inputs["kp_sw"], dtype=np.float32)
    kp_sb = np.ascontiguousarray(inputs["kp_sb"], dtype=np.float32)
    kp_pw = np.ascontiguousarray(inputs["kp_pw"], dtype=np.float32)
    kp_pb = np.ascontiguousarray(inputs["kp_pb"], dtype=np.float32)
    kp_bw = np.ascontiguousarray(inputs["kp_bw"], dtype=np.float32)
    kp_bb = np.ascontiguousarray(inputs["kp_bb"], dtype=np.float32)
    dec_w1 = np.ascontiguousarray(inputs["dec_w1"], dtype=np.float32)
    dec_b1 = np.ascontiguousarray(inputs["dec_b1"], dtype=np.float32)
    dec_w2 = np.ascontiguousarray(inputs["dec_w2"], dtype=np.float32)
    dec_b2 = np.ascontiguousarray(inputs["dec_b2"], dtype=np.float32)

    import ml_dtypes

    bf16 = ml_dtypes.bfloat16

    # column permutation p' = (t, g, i, co) -> o = (t*128 + g*8 + co)*8 + i
    O = np.arange(C * CG).reshape(NT, 16, CG, CG)  # indexed (t, g, co, i), o-major
    P = O.transpose(0, 1, 3, 2).reshape(-1)        # (t, g, i, co)

    kpsw = np.empty((9 * S + 1, C * CG), dtype=np.float32)
    kpsw[: 9 * S] = (
        kp_sw[P].reshape(C * CG, S, 3, 3).transpose(2, 3, 1, 0).reshape(9 * S, C * CG)
    )  # rows in k-order (di, dj, s)
    kpsw[9 * S] = kp_sb[P]

    # pw column order: o_pw = (g_glob, co, i) flat -> p' = (t, g, i, co)
    O2 = np.arange(C * CG).reshape(NT, 16, CG, CG)  # (t, g, co, i), opw-major
    P2 = O2.transpose(0, 1, 3, 2).reshape(-1)
    kppw = np.empty((S + 1, C * CG), dtype=np.float32)
    kppw[:S] = kp_pw[P2].T
    kppw[S] = kp_pb[P2]

    kpbw = np.empty((S + 1, C), dtype=np.float32)
    kpbw[:S] = kp_bw.T
    kpbw[S] = kp_bb

    w1t = np.ascontiguousarray(dec_w1.transpose(2, 3, 1, 0).reshape(9, C, C))
    w2t = np.ascontiguousarray(dec_w2.transpose(2, 3, 1, 0).reshape(9, C, CO))

    shared = {
        "kpbw": kpbw.astype(bf16),
        "w1t": w1t.astype(bf16),
        "w2t": w2t.astype(bf16),
        "b1d": dec_b1,
        "b2d": dec_b2,
    }
    kpsw_h = kpsw.astype(bf16)
    kppw_h = kppw.astype(bf16)
    percore = []
    for c in range(NCORES):
        sl = slice(SHARD * c, SHARD * (c + 1))
        percore.append(
            {
                "kpswc": np.ascontiguousarray(kpsw_h[:, sl]),
                "kppwc": np.ascontiguousarray(kppw_h[:, sl]),
            }
        )
    return shared, percore


def _make_in_maps(x, w, repacked):
    shared, percore = repacked
    in_maps = []
    for c in range(NCORES):
        sl = slice(BPC * c, BPC * (c + 1))
        in_maps.append(
            {"x2": x[sl], "wst": w[sl], "wfull": w, **shared, **percore[c]}
        )
    return in_maps


def kernel(**inputs):
    if "nc" not in _CACHE:
        _CACHE["nc"] = _build()
    nc = _CACHE["nc"]

    import ml_dtypes

    repacked = _repack(inputs)
    x = np.ascontiguousarray(inputs["x"]).astype(ml_dtypes.bfloat16)
    w = np.ascontiguousarray(inputs["w"], dtype=np.float32)

    in_maps = _make_in_maps(x, w, repacked)
    res = run_bass_kernel_spmd(nc, in_maps, list(range(NCORES))).results
    return np.concatenate([r["yout"] for r in res], axis=0)
